# revision 9
# baseline (speedup 1.0000x reference)
"""Trainium2 Bass kernel for GNMT-style seq2seq LSTM with attention.

Strategy (8 NeuronCores, SPMD identical program, per-core data shards):
- LSTM gate matrices H-sharded column-wise 8 ways (512 gate-cols/core).
- Encoder and decoder recurrences run interleaved on all 8 cores; each
  step does one merged AllGather per stream carrying [h1(p).T, h2(p-1).T]
  (decoupled lag-1 pipeline: L2 runs one step behind L1).
- Layer-1 input projections are precomputed as one batched GEMM via the
  fused matrix F = proj_W_in @ W1x (so x_t never materializes).
- Attention + W_c + proj + vocab projection + softmax are fully batched
  after the recurrences: scores H-sharded partial + AllReduce, ctx
  H-sharded + AllGather, outemb partial + AllReduce, Wo vocab-sharded.
"""
import numpy as np
import jax
from jax.sharding import Mesh, PartitionSpec
from jax.experimental.shard_map import shard_map

import concourse.bass as bass  # noqa: F401
import concourse.mybir as mybir
import concourse.tile as tile
from concourse import bacc, bass2jax
from concourse.masks import make_identity

F32 = mybir.dt.float32
NC = 8
B, E, H, VT = 32, 512, 1024, 32000
VSH = VT // NC          # vocab shard 4000
GSH = 4 * H // NC       # gate-col shard 512
HSH = H // NC           # h-slice 128
AF = mybir.ActivationFunctionType
ALU = mybir.AluOpType
RG_ALL = [list(range(NC))]


def _build_program(T):
    TB = T * B
    MT = TB // 128                      # tb-tiles of 128
    NTSZ = min(512, TB)                 # tb GEMM tile width
    NT_TB = TB // NTSZ
    nc = bacc.Bacc("TRN2", target_bir_lowering=False, debug=False,
                   num_devices=NC)

    def din(name, shape):
        return nc.dram_tensor(name, shape, F32, kind="ExternalInput").ap()

    embS_T = din("embS_T", [E, TB])
    embT_T = din("embT_T", [E, TB])
    spWT = din("spWT", [H, E])
    tpWT = din("tpWT", [H, E])
    w1x_e = din("w1x_e", [H, GSH])
    w1h_e = din("w1h_e", [H, GSH])
    w2_e = din("w2_e", [2 * H, GSH])
    w1x_d = din("w1x_d", [H, GSH])
    w1h_d = din("w1h_d", [H, GSH])
    w2_d = din("w2_d", [2 * H, GSH])
    zb1_e = din("zb1_e", [1, GSH])
    zb2_e = din("zb2_e", [1, GSH])
    zb1_d = din("zb1_d", [1, GSH])
    zb2_d = din("zb2_d", [1, GSH])
    wc_sh = din("wc_sh", [2 * H, HSH])
    bc_sh = din("bc_sh", [1, HSH])
    pw_sh = din("pw_sh", [HSH, E])
    pb8 = din("pb8", [1, E])
    wo_sh = din("wo_sh", [E, VSH])
    bo_sh = din("bo_sh", [1, VSH])
    probs = nc.dram_tensor("probs", [TB, VSH], F32, kind="ExternalOutput").ap()

    with tile.TileContext(nc) as tc:
        with (
            tc.tile_pool(name="const", bufs=1) as cpool,
            tc.tile_pool(name="ps", bufs=8, space="PSUM") as ps,
            tc.tile_pool(name="state", bufs=1) as state,
            tc.tile_pool(name="bncp", bufs=8, space="DRAM") as bncp,
            tc.tile_pool(name="dfix", bufs=1, space="DRAM") as dfix,
        ):
            ident32 = cpool.tile([32, 32], F32, tag="i32")
            make_identity(nc, ident32[:])
            ident64 = cpool.tile([64, 64], F32, tag="i64")
            make_identity(nc, ident64[:])
            ident128 = cpool.tile([128, 128], F32, tag="i128")
            make_identity(nc, ident128[:])
            ones = cpool.tile([1, 512], F32, tag="ones")
            nc.vector.memset(ones[:], 1.0)

            zx_dram = {s: dfix.tile([TB, GSH], F32, name=f"zx{s}", tag=f"zx{s}")
                       for s in "ed"}
            hist = {s: dfix.tile([T + 2, NC, 2, 128, 32], F32,
                                 name=f"hist{s}", tag=f"hist{s}") for s in "ed"}
            hT_hist = {s: state.tile([128, T, 32], F32, name=f"hh{s}", tag=f"hh{s}")
                       for s in "ed"}

            # ---------------- Phase 1: Zx precompute --------------------
            with tc.tile_pool(name="p1", bufs=3) as p1, \
                 tc.tile_pool(name="p1w", bufs=1) as p1w:
                for s, embt, pwt, w1x, zb1 in (
                        ("e", embS_T, spWT, w1x_e, zb1_e),
                        ("d", embT_T, tpWT, w1x_d, zb1_d)):
                    emb_sb = p1w.tile([128, 4, TB], F32, name=f"emb{s}", tag=f"emb{s}")
                    nc.sync.dma_start(
                        out=emb_sb[:],
                        in_=embt.rearrange("(c p) t -> p c t", p=128))
                    pwt_sb = p1w.tile([128, 8, E], F32, name=f"pwt{s}", tag=f"pwt{s}")
                    nc.sync.dma_start(
                        out=pwt_sb[:],
                        in_=pwt.rearrange("(c p) e -> p c e", p=128))
                    w1x_sb = p1w.tile([128, 8, GSH], F32, name=f"w1x{s}", tag=f"w1x{s}")
                    nc.sync.dma_start(
                        out=w1x_sb[:],
                        in_=w1x.rearrange("(c p) n -> p c n", p=128))
                    zb1_sb = p1w.tile([1, GSH], F32, name=f"zb{s}", tag=f"zb{s}")
                    nc.sync.dma_start(out=zb1_sb[:], in_=zb1[:])

                    # F = spW @ W1x  [E, GSH]
                    f_sb = p1w.tile([128, 4, GSH], F32, name=f"f{s}", tag=f"f{s}")
                    for m in range(4):
                        fp = ps.tile([128, GSH], F32, tag="bank")
                        for c in range(8):
                            nc.tensor.matmul(
                                fp[:], pwt_sb[:, c, m * 128:(m + 1) * 128],
                                w1x_sb[:, c, :], start=(c == 0), stop=(c == 7))
                        nc.vector.tensor_copy(f_sb[:, m, :], fp[:])
                    # Zx = emb-chunks @ F + zb1
                    for mt in range(MT):
                        zp = ps.tile([128, GSH], F32, tag="bank")
                        for c in range(4):
                            nc.tensor.matmul(
                                zp[:], emb_sb[:, c, mt * 128:(mt + 1) * 128],
                                f_sb[:, c, :], start=(c == 0), stop=False)
                        nc.tensor.matmul(zp[:], ones[:, :128], zb1_sb[:],
                                         start=False, stop=True)
                        zx_sb = p1.tile([128, GSH], F32, tag="zxsb")
                        nc.vector.tensor_copy(zx_sb[:], zp[:])
                        nc.sync.dma_start(
                            out=zx_dram[s][mt * 128:(mt + 1) * 128, :],
                            in_=zx_sb[:])

            # ---------------- Phase 2: recurrences ----------------------
            with (
                tc.tile_pool(name="wts", bufs=1) as wts,
                tc.tile_pool(name="loop", bufs=4) as loop,
                tc.tile_pool(name="gates", bufs=12) as gates,
            ):
                w_sb = {}
                for s, w1h, w2, zb2 in (("e", w1h_e, w2_e, zb2_e),
                                        ("d", w1h_d, w2_d, zb2_d)):
                    w_sb[s, "w1h"] = wts.tile([128, 8, GSH], F32,
                                              name=f"w1h{s}", tag=f"w1h{s}")
                    nc.sync.dma_start(
                        out=w_sb[s, "w1h"][:],
                        in_=w1h.rearrange("(c p) n -> p c n", p=128))
                    w_sb[s, "w2"] = wts.tile([128, 16, GSH], F32,
                                             name=f"w2{s}", tag=f"w2{s}")
                    nc.sync.dma_start(
                        out=w_sb[s, "w2"][:],
                        in_=w2.rearrange("(c p) n -> p c n", p=128))
                    w_sb[s, "zb2"] = wts.tile([1, GSH], F32, name=f"zb2{s}", tag=f"zb2{s}")
                    nc.sync.dma_start(out=w_sb[s, "zb2"][:], in_=zb2[:])

                cstate = {}
                for s in "ed":
                    for lay in (1, 2):
                        ct = wts.tile([32, 128], F32, name=f"c{lay}{s}", tag=f"c{lay}{s}")
                        nc.vector.memset(ct[:], 0.0)
                        cstate[s, lay] = ct
                zchunks = wts.tile([128, NC, 2, 32], F32, tag="zch")
                nc.vector.memset(zchunks[:], 0.0)

                def lstm_nonlin(z_ap, c_tile, h_tile):
                    """z_ap [32, GSH] (i|j|f|o each 128); updates c, writes h."""
                    si = gates.tile([32, 128], F32, tag="gsi")
                    tj = gates.tile([32, 128], F32, tag="gtj")
                    sf = gates.tile([32, 128], F32, tag="gsf")
                    so = gates.tile([32, 128], F32, tag="gso")
                    th = gates.tile([32, 128], F32, tag="gth")
                    nc.scalar.activation(si[:], z_ap[:, 0:128], AF.Sigmoid)
                    nc.scalar.activation(tj[:], z_ap[:, 128:256], AF.Tanh)
                    nc.scalar.activation(sf[:], z_ap[:, 256:384], AF.Sigmoid,
                                         bias=1.0)
                    nc.scalar.activation(so[:], z_ap[:, 384:512], AF.Sigmoid)
                    nc.vector.tensor_mul(c_tile[:], c_tile[:], sf[:])
                    nc.vector.tensor_mul(si[:], si[:], tj[:])
                    nc.vector.tensor_add(c_tile[:], c_tile[:], si[:])
                    nc.scalar.activation(th[:], c_tile[:], AF.Tanh)
                    nc.vector.tensor_mul(h_tile[:], th[:], so[:])

                for p in range(T + 1):
                    for s in "ed":
                        if p == 0:
                            hch = zchunks
                        else:
                            hch = loop.tile([128, NC, 2, 32], F32, tag="hch")
                            nc.sync.dma_start(
                                out=hch[:],
                                in_=hist[s][p].rearrange("c l p b -> p c l b"))
                        trp = ps.tile([128, 2, 32], F32, tag="bank")
                        if p < T:
                            zx_t = loop.tile([32, GSH], F32, tag="zxt")
                            nc.sync.dma_start(
                                out=zx_t[:],
                                in_=zx_dram[s][p * 32:(p + 1) * 32, :])
                            ps1 = ps.tile([32, GSH], F32, tag="bank")
                            for c in range(8):
                                nc.tensor.matmul(
                                    ps1[:], hch[:, c, 0, :],
                                    w_sb[s, "w1h"][:, c, :],
                                    start=(c == 0), stop=(c == 7))
                            z1 = gates.tile([32, GSH], F32, tag="z1")
                            nc.vector.tensor_add(z1[:], ps1[:], zx_t[:])
                            h1 = gates.tile([32, 128], F32, tag="h1")
                            lstm_nonlin(z1, cstate[s, 1], h1)
                            nc.tensor.transpose(trp[:, 0, :], h1[:],
                                                ident32[:])
                        else:
                            nc.vector.memset(trp[:, 0, :], 0.0)
                        if p > 0:
                            ps2 = ps.tile([32, GSH], F32, tag="bank")
                            for c in range(8):
                                nc.tensor.matmul(
                                    ps2[:], hch[:, c, 0, :],
                                    w_sb[s, "w2"][:, c, :],
                                    start=(c == 0), stop=False)
                            for c in range(8):
                                nc.tensor.matmul(
                                    ps2[:], hch[:, c, 1, :],
                                    w_sb[s, "w2"][:, 8 + c, :],
                                    start=False, stop=False)
                            nc.tensor.matmul(ps2[:], ones[:, :32],
                                             w_sb[s, "zb2"][:],
                                             start=False, stop=True)
                            h2 = gates.tile([32, 128], F32, tag="h2")
                            lstm_nonlin(ps2, cstate[s, 2], h2)
                            nc.tensor.transpose(trp[:, 1, :], h2[:],
                                                ident32[:])
                        else:
                            nc.vector.memset(trp[:, 1, :], 0.0)
                        pair = loop.tile([128, 2, 32], F32, tag="pair")
                        nc.vector.tensor_copy(pair[:], trp[:])
                        if p > 0:
                            nc.vector.tensor_copy(hT_hist[s][:, p - 1, :],
                                                  pair[:, 1, :])
                        bounce = bncp.tile([2, 128, 32], F32, tag="bnc")
                        nc.sync.dma_start(
                            out=bounce[:].rearrange("l p b -> p l b"),
                            in_=pair[:])
                        nc.gpsimd.collective_compute(
                            "AllGather", ALU.bypass, replica_groups=RG_ALL,
                            ins=[bounce[:].opt()],
                            outs=[hist[s][p + 1].opt()])

            # ---------------- Phase 3: batched tail ---------------------
            with (
                tc.tile_pool(name="att", bufs=1) as att,
                tc.tile_pool(name="t3", bufs=2) as t3,
                tc.tile_pool(name="xhp", bufs=2) as xhp,
                tc.tile_pool(name="big", bufs=1) as big,
            ):
                # scores partial: per b, [T(dec) x T(enc)] over my h-slice
                sc_sb = att.tile([T, 32, T], F32, tag="sc")
                for b in range(32):
                    scp = ps.tile([T, T], F32, tag="bank")
                    nc.tensor.matmul(scp[:], hT_hist["d"][:, :, b],
                                     hT_hist["e"][:, :, b],
                                     start=True, stop=True)
                    nc.vector.tensor_copy(sc_sb[:, b, :], scp[:])
                scb_in = bncp.tile([T, 32, T], F32, tag="scin")
                scb_out = bncp.tile([T, 32, T], F32, tag="scout")
                nc.sync.dma_start(out=scb_in[:], in_=sc_sb[:])
                nc.gpsimd.collective_compute(
                    "AllReduce", ALU.add, replica_groups=RG_ALL,
                    ins=[scb_in[:].opt()], outs=[scb_out[:].opt()])
                nc.sync.dma_start(out=sc_sb[:], in_=scb_out[:])

                # softmax over t_enc; transpose attn & enc per b; ctx
                ctxT = att.tile([128, T, 32], F32, tag="ctxT")
                for b in range(32):
                    nm = t3.tile([T, 1], F32, tag="nm")
                    nc.vector.tensor_reduce(
                        out=nm[:], in_=sc_sb[:, b, :], op=ALU.max,
                        axis=mybir.AxisListType.X, negate=True)
                    at = t3.tile([T, T], F32, tag="at")
                    es = t3.tile([T, 1], F32, tag="es")
                    nc.scalar.activation(at[:], sc_sb[:, b, :], AF.Exp,
                                         bias=nm[:], accum_out=es[:])
                    inv = t3.tile([T, 1], F32, tag="inv")
                    nc.vector.reciprocal(inv[:], es[:])
                    nc.vector.tensor_scalar_mul(at[:], at[:], inv[:])
                    atp = ps.tile([T, T], F32, tag="bank")
                    nc.tensor.transpose(atp[:], at[:], ident64[:T, :T])
                    atT = t3.tile([T, T], F32, tag="atT")
                    nc.vector.tensor_copy(atT[:], atp[:])
                    etp = ps.tile([T, 128], F32, tag="bank")
                    nc.tensor.transpose(etp[:], hT_hist["e"][:, :, b],
                                        ident128[:])
                    etT = t3.tile([T, 128], F32, tag="etT")
                    nc.vector.tensor_copy(etT[:], etp[:])
                    ctp = ps.tile([128, T], F32, tag="bank")
                    nc.tensor.matmul(ctp[:], etT[:], atT[:],
                                     start=True, stop=True)
                    nc.vector.tensor_copy(ctxT[:, :, b], ctp[:])

                # AllGather ctx.T -> full [NC, 128, T, 32]
                ctxb_in = bncp.tile([128, T, 32], F32, tag="ctxin")
                ctx_full = dfix.tile([NC, 128, T, 32], F32, tag="ctxfull")
                nc.sync.dma_start(out=ctxb_in[:], in_=ctxT[:])
                nc.gpsimd.collective_compute(
                    "AllGather", ALU.bypass, replica_groups=RG_ALL,
                    ins=[ctxb_in[:].opt()], outs=[ctx_full[:].opt()])

                # W_c: h_t.T [my 128-slice, TB]
                wc_sb = big.tile([128, 16, HSH], F32, tag="wc")
                nc.sync.dma_start(
                    out=wc_sb[:],
                    in_=wc_sh.rearrange("(c p) n -> p c n", p=128))
                bc_sb = big.tile([1, HSH], F32, tag="bc")
                nc.sync.dma_start(out=bc_sb[:], in_=bc_sh[:])
                htp = [ps.tile([128, NTSZ], F32, name=f"htp{i}", tag="bank")
                       for i in range(NT_TB)]
                for c in range(16):
                    xh = xhp.tile([128, TB], F32, tag="xh")
                    if c < 8:
                        nc.sync.dma_start(
                            out=xh[:].rearrange("p (t b) -> p t b", b=32),
                            in_=hist["d"][2:T + 2, c, 1]
                            .rearrange("t p b -> p t b"))
                    else:
                        nc.sync.dma_start(
                            out=xh[:],
                            in_=ctx_full[c - 8].rearrange("p t b -> p (t b)"))
                    for nt in range(NT_TB):
                        nc.tensor.matmul(
                            htp[nt][:], wc_sb[:, c, :],
                            xh[:, nt * NTSZ:(nt + 1) * NTSZ],
                            start=(c == 0), stop=False)
                ht_sb = big.tile([128, TB], F32, tag="ht")
                for nt in range(NT_TB):
                    nc.tensor.matmul(htp[nt][:], bc_sb[:], ones[:, :NTSZ],
                                     start=False, stop=True)
                    nc.vector.tensor_copy(
                        ht_sb[:, nt * NTSZ:(nt + 1) * NTSZ], htp[nt][:])

                # proj: partial outemb.T [E, TB] (contraction over my slice),
                # streamed straight to the AllReduce bounce in DRAM
                pw_sb = big.tile([128, E], F32, tag="pw")
                nc.sync.dma_start(out=pw_sb[:], in_=pw_sh[:])
                pb8_sb = big.tile([1, E], F32, tag="pb8")
                nc.sync.dma_start(out=pb8_sb[:], in_=pb8[:])
                oeb_in = bncp.tile([128, 4, TB], F32, tag="oein")
                oeb_out = bncp.tile([128, 4, TB], F32, tag="oeout")
                for m in range(4):
                    for nt in range(NT_TB):
                        op_ = ps.tile([128, NTSZ], F32, tag="bank")
                        nc.tensor.matmul(
                            op_[:], pw_sb[:, m * 128:(m + 1) * 128],
                            ht_sb[:, nt * NTSZ:(nt + 1) * NTSZ],
                            start=True, stop=False)
                        nc.tensor.matmul(
                            op_[:], pb8_sb[:, m * 128:(m + 1) * 128],
                            ones[:, :NTSZ], start=False, stop=True)
                        oe_st = t3.tile([128, NTSZ], F32, tag="oest")
                        nc.vector.tensor_copy(oe_st[:], op_[:])
                        nc.sync.dma_start(
                            out=oeb_in[:, m, nt * NTSZ:(nt + 1) * NTSZ],
                            in_=oe_st[:])
                nc.gpsimd.collective_compute(
                    "AllReduce", ALU.add, replica_groups=RG_ALL,
                    ins=[oeb_in[:].opt()], outs=[oeb_out[:].opt()])

                # Wo + softmax, vocab shard
                wo_sb = big.tile([128, 4, VSH], F32, tag="wo")
                nc.sync.dma_start(
                    out=wo_sb[:],
                    in_=wo_sh.rearrange("(c p) v -> p c v", p=128))
                bo_sb = big.tile([1, VSH], F32, tag="bo")
                nc.sync.dma_start(out=bo_sb[:], in_=bo_sh[:])
                # pass A: logits -> exp (logits are O(1), exp is safe
                # unnormalized), spill exp to DRAM, partial sums per core
                exps_dram = dfix.tile([MT, 128, 8, 500], F32, name="exps",
                                      tag="exps")
                ssum = big.tile([128, MT], F32, tag="ssum")
                for mt in range(MT):
                    oe_mt = t3.tile([128, 4, 128], F32, tag="oemt")
                    nc.sync.dma_start(
                        out=oe_mt[:],
                        in_=oeb_out[:, :, mt * 128:(mt + 1) * 128])
                    lg = t3.tile([128, 8, 500], F32, tag="lg")
                    for nt in range(8):
                        lp = ps.tile([128, 512], F32, tag="bank")
                        for ek in range(4):
                            nc.tensor.matmul(
                                lp[:, :500],
                                oe_mt[:, ek, :],
                                wo_sb[:, ek, nt * 500:(nt + 1) * 500],
                                start=(ek == 0), stop=False)
                        nc.tensor.matmul(
                            lp[:, :500], ones[:, :128],
                            bo_sb[:, nt * 500:(nt + 1) * 500],
                            start=False, stop=True)
                        nc.vector.tensor_copy(lg[:, nt, :], lp[:, :500])
                    nc.scalar.activation(lg[:], lg[:], AF.Exp,
                                         accum_out=ssum[:, mt:mt + 1])
                    nc.sync.dma_start(out=exps_dram[mt], in_=lg[:])
                # global sum across vocab shards
                ssb_in = bncp.tile([128, MT], F32, tag="ssin")
                ssb_out = bncp.tile([128, MT], F32, tag="ssout")
                nc.sync.dma_start(out=ssb_in[:], in_=ssum[:])
                nc.gpsimd.collective_compute(
                    "AllReduce", ALU.add, replica_groups=RG_ALL,
                    ins=[ssb_in[:].opt()], outs=[ssb_out[:].opt()])
                nc.sync.dma_start(out=ssum[:], in_=ssb_out[:])
                sinv = big.tile([128, MT], F32, tag="sinv")
                nc.vector.reciprocal(sinv[:], ssum[:])
                # pass B: scale and emit
                for mt in range(MT):
                    et = t3.tile([128, 8, 500], F32, tag="lg")
                    nc.sync.dma_start(out=et[:], in_=exps_dram[mt])
                    nc.vector.tensor_scalar_mul(et[:], et[:],
                                                sinv[:, mt:mt + 1])
                    nc.sync.dma_start(
                        out=probs[mt * 128:(mt + 1) * 128, :]
                        .rearrange("p (nt v) -> p nt v", nt=8),
                        in_=et[:])
    nc.compile()
    return nc


# --------------------------------------------------------------------------
# Host side
# --------------------------------------------------------------------------

def _prep_inputs(inputs, T):
    src = np.asarray(inputs["source"])[:, :T]
    tgt = np.asarray(inputs["target"])[:, :T]
    s_emb = np.asarray(inputs["s_emb"], np.float32)
    t_emb = np.asarray(inputs["t_emb"], np.float32)
    embS_T = np.ascontiguousarray(
        s_emb[src].transpose(1, 0, 2).reshape(T * B, E).T)
    embT_T = np.ascontiguousarray(
        t_emb[tgt].transpose(1, 0, 2).reshape(T * B, E).T)
    spW = np.asarray(inputs["s_proj_W"], np.float32)
    tpW = np.asarray(inputs["t_proj_W"], np.float32)
    spb = np.asarray(inputs["s_proj_b"], np.float32)
    tpb = np.asarray(inputs["t_proj_b"], np.float32)
    encW = np.asarray(inputs["enc_W"], np.float32)
    encb = np.asarray(inputs["enc_b"], np.float32)
    decW = np.asarray(inputs["dec_W"], np.float32)
    decb = np.asarray(inputs["dec_b"], np.float32)
    W_c = np.asarray(inputs["W_c"], np.float32)
    b_c = np.asarray(inputs["b_c"], np.float32)
    pW = np.asarray(inputs["proj_W"], np.float32)
    pb = np.asarray(inputs["proj_b"], np.float32)
    Wo = np.asarray(inputs["proj_Wo"], np.float32)
    bo = np.asarray(inputs["proj_bo"], np.float32)

    spWT = np.ascontiguousarray(spW.T)
    tpWT = np.ascontiguousarray(tpW.T)
    in_maps = []
    for k in range(NC):
        cols = np.concatenate(
            [np.arange(g * H + k * HSH, g * H + (k + 1) * HSH)
             for g in range(4)])
        m = dict(
            embS_T=embS_T, embT_T=embT_T, spWT=spWT, tpWT=tpWT,
            w1x_e=encW[0][:H, cols],
            w1h_e=encW[0][H:, cols],
            w2_e=encW[1][:, cols],
            w1x_d=decW[0][:H, cols],
            w1h_d=decW[0][H:, cols],
            w2_d=decW[1][:, cols],
            zb1_e=(spb @ encW[0][:H, cols] + encb[0][cols])[None, :],
            zb2_e=encb[1][cols][None, :],
            zb1_d=(tpb @ decW[0][:H, cols] + decb[0][cols])[None, :],
            zb2_d=decb[1][cols][None, :],
            wc_sh=W_c[:, k * HSH:(k + 1) * HSH],
            bc_sh=b_c[k * HSH:(k + 1) * HSH][None, :],
            pw_sh=pW[k * HSH:(k + 1) * HSH, :],
            pb8=(pb / NC)[None, :],
            wo_sh=Wo[:, k * VSH:(k + 1) * VSH],
            bo_sh=bo[k * VSH:(k + 1) * VSH][None, :],
        )
        in_maps.append({n: np.ascontiguousarray(v, dtype=np.float32)
                        for n, v in m.items()})
    return in_maps


class _Runner:
    """Persistent-jit SPMD runner (axon PJRT path, keeps compiled fn)."""

    def __init__(self, nc, n_cores):
        bass2jax.install_neuronx_cc_hook()
        self.n_cores = n_cores
        partition_name = (nc.partition_id_tensor.name
                          if nc.partition_id_tensor else None)
        in_names, out_names, out_avals = [], [], []
        for alloc in nc.m.functions[0].allocations:
            if not isinstance(alloc, mybir.MemoryLocationSet):
                continue
            name = alloc.memorylocations[0].name
            if alloc.kind == "ExternalInput":
                if name != partition_name:
                    in_names.append(name)
            elif alloc.kind == "ExternalOutput":
                out_names.append(name)
                out_avals.append(jax.core.ShapedArray(
                    tuple(alloc.tensor_shape), mybir.dt.np(alloc.dtype)))
        self.in_names, self.out_names, self.out_avals = \
            in_names, out_names, out_avals
        n_params, n_outs = len(in_names), len(out_avals)
        all_in = list(in_names) + list(out_names)
        if partition_name is not None:
            all_in.append(partition_name)

        def _body(*args):
            operands = list(args)
            if partition_name is not None:
                operands.append(bass2jax.partition_id_tensor())
            return tuple(bass2jax._bass_exec_p.bind(
                *operands, out_avals=tuple(out_avals),
                in_names=tuple(all_in), out_names=tuple(out_names),
                lowering_input_output_aliases=(),
                sim_require_finite=True, sim_require_nnan=True, nc=nc))

        devices = jax.devices()[:n_cores]
        mesh = Mesh(np.asarray(devices), ("core",))
        self._fn = jax.jit(
            shard_map(_body, mesh=mesh,
                      in_specs=(PartitionSpec("core"),) * (n_params + n_outs),
                      out_specs=(PartitionSpec("core"),) * n_outs,
                      check_rep=False),
            donate_argnums=tuple(range(n_params, n_params + n_outs)),
            keep_unused=True)

    def run(self, in_maps):
        ncr = self.n_cores
        concat_in = [np.concatenate([np.asarray(m[nm]) for m in in_maps],
                                    axis=0) for nm in self.in_names]
        concat_zeros = [np.zeros((ncr * a.shape[0], *a.shape[1:]), a.dtype)
                        for a in self.out_avals]
        outs = jax.block_until_ready(self._fn(*concat_in, *concat_zeros))
        return [
            {nm: np.asarray(outs[i]).reshape(ncr, *self.out_avals[i].shape)[c]
             for i, nm in enumerate(self.out_names)}
            for c in range(ncr)
        ]


_CACHE = {}


def _get_runner(T):
    if T not in _CACHE:
        _CACHE[T] = _Runner(_build_program(T), NC)
    return _CACHE[T]


def kernel(_T=64, **inputs):
    T = _T
    runner = _get_runner(T)
    in_maps = _prep_inputs(inputs, T)
    res = runner.run(in_maps)
    out = np.empty((B, T, VT), np.float32)
    for k in range(NC):
        blk = res[k]["probs"].reshape(T, B, VSH)
        out[:, :, k * VSH:(k + 1) * VSH] = blk.transpose(1, 0, 2)
    return out
